# revision 19
# baseline (speedup 1.0000x reference)
"""Entmax (alpha=1.25) bisection kernel for Trainium2, 8 NeuronCores.

Solves  sum_j relu(x_j - tau)^4 = 256  per row (tau = 4*tau_ref) and emits
p = relu(x - tau)^4 / P.  Per 128-row tile:
  1. stream the row in as f32; GPSIMD down-converts to fp16 (x~); the
     scheme then solves entmax of x~ exactly (the fp16 rounding of x
     contributes ~3e-3 relative error, well inside the 2e-2 gate),
  2. group maxima cmax[250] (groups of 128) via a pairwise tt-max tree on
     fp16 (2x DVE mode), 4 levels per chunk + 3 tile-wide,
  3. bracket tau with 2 rounds of 7-point parallel bisection on
     G(t) = sum relu(cmax - t)^4: the 7 G evaluations per round are
     independent (short critical path); lo += step * #{G_k >= 256}.
     G <= F pointwise so theta = lo - margin is a guaranteed lower bound
     of tau, within d <~ 0.14,
  4. moment pass at theta: u = relu(x~-theta) (DVE 4x ts), z = u^2 + A2
     (ACT square-accum), A3 = sum z*u (DVE STT-accum), A4 = sum z^2 (ACT
     square-accum),
  5. 2 Newton steps on P(d) = A4 - 4*A3 d + 6*A2 d^2 = 256 give
     tau = theta + d and the normalizer P,
  6. output: v = relu(u - d) (== relu(x~-tau) exactly; frees x~ early so
     the next tile's loads overlap this tile's moment phase), q = v^2
     (GPSIMD mul, 3/10 chunks on DVE), p = (s2*q)^2 on ACT with
     s2 = P^-1/2 (f32 out) -> DMA.

Engine totals per core (us): DVE ~180 (tree, bisect, u, v, A3, 3 q),
ACT ~179 (z, A4, p), Pool ~180 (conv, 7 q), DMA ~182 (65.5 MB at
360 B/ns) -- balanced against the DMA roofline.
"""

import numpy as np

import concourse.bass as bass
import concourse.mybir as mybir
from concourse.tile import TileContext

P = 128                    # partitions
D = 32000                  # row length
ROWS_PER_CORE = 256        # 2048 / 8 cores
N_ROW_TILES = ROWS_PER_CORE // P   # 2
N_CORES = 8

CHUNK = 3200               # column chunk (load, moments, output)
N_CHUNKS = D // CHUNK      # 10
CW = 128                   # elements per chunk-max group
NG = CHUNK // CW           # 25 groups per chunk
CMAX_W = D // CW           # 250 group maxima per row tile
B_ROUNDS = 2               # parallel-bisection rounds
B_K = 7                    # candidates per round (3 bits)
G_MARGIN = 0.002           # fp16 G-evaluation safety margin on theta
NEWTON_ITERS = 2

F32 = mybir.dt.float32
FP16 = mybir.dt.float16
BF16 = mybir.dt.bfloat16
DM0 = float(np.float32(4.0 - 4.0 * (1.0 / D) ** 0.25))  # initial bracket width


def _row_tile(tc, pools, x_dram, out_dram, sv_sb, row0, tidx):
    nc = tc.nc
    xt, fio, fout, wk, tr1, tr2, tr3, t4p, trs, bis, small = pools
    Alu = mybir.AluOpType
    Act = mybir.ActivationFunctionType

    # ---- load, fp16 convert (GPSIMD), per-chunk max-tree levels 1-4 ----
    xts = []
    T4 = t4p.tile([P, N_CHUNKS * NG * 8], FP16, tag="t4")   # [P, 2000]
    rmaxp = small.tile([P, N_CHUNKS], F32, tag="rmaxp")
    for c in range(N_CHUNKS):
        f_in = fio.tile([P, CHUNK], F32, tag="fin", name=f"fin{c}")
        nc.sync.dma_start(
            out=f_in, in_=x_dram[row0 : row0 + P, c * CHUNK : (c + 1) * CHUNK]
        )
        x_c = xt.tile([P, CHUNK], FP16, tag="xt", name=f"xt{c}")
        if tidx == 0 and c % 2 == 1:
            nc.vector.tensor_copy(x_c, f_in)
        else:
            nc.gpsimd.tensor_copy(x_c, f_in)
        xts.append(x_c)

        g0 = x_c.rearrange("p (a b) -> p a b", b=CW)            # [P, 25, 128]
        with tc.high_priority():
            l1 = tr1.tile([P, NG * 64], FP16, tag="tr1")
            l1v = l1.rearrange("p (a b) -> p a b", b=64)
            nc.vector.tensor_tensor(
                out=l1v, in0=g0[:, :, 0:64], in1=g0[:, :, 64:128], op=Alu.max
            )
            l2 = tr2.tile([P, NG * 32], FP16, tag="tr2")
            l2v = l2.rearrange("p (a b) -> p a b", b=32)
            nc.vector.tensor_tensor(
                out=l2v, in0=l1v[:, :, 0:32], in1=l1v[:, :, 32:64], op=Alu.max
            )
            l3 = tr3.tile([P, NG * 16], FP16, tag="tr3")
            l3v = l3.rearrange("p (a b) -> p a b", b=16)
            nc.vector.tensor_tensor(
                out=l3v, in0=l2v[:, :, 0:16], in1=l2v[:, :, 16:32], op=Alu.max
            )
            t4v = T4[:, c * NG * 8 : (c + 1) * NG * 8].rearrange(
                "p (a b) -> p a b", b=8
            )
            nc.vector.tensor_tensor(
                out=t4v, in0=l3v[:, :, 0:8], in1=l3v[:, :, 8:16], op=Alu.max
            )
            nc.vector.reduce_max(
                out=rmaxp[:, c : c + 1],
                in_=T4[:, c * NG * 8 : (c + 1) * NG * 8],
                axis=mybir.AxisListType.X,
            )

    # ---- tile-wide max-tree levels 5-7 -> cmax [P, 250] ----
    with tc.high_priority():
        T4v = T4.rearrange("p (a b) -> p a b", b=8)             # [P, 250, 8]
        T5 = trs.tile([P, CMAX_W * 4], FP16, tag="t5")
        T5v = T5.rearrange("p (a b) -> p a b", b=4)
        nc.vector.tensor_tensor(
            out=T5v, in0=T4v[:, :, 0:4], in1=T4v[:, :, 4:8], op=Alu.max
        )
        T6 = trs.tile([P, CMAX_W * 2], FP16, tag="t6")
        T6v = T6.rearrange("p (a b) -> p a b", b=2)
        nc.vector.tensor_tensor(
            out=T6v, in0=T5v[:, :, 0:2], in1=T5v[:, :, 2:4], op=Alu.max
        )
        cmax = trs.tile([P, CMAX_W], FP16, tag="cmax")
        cmaxv = cmax.rearrange("p (a b) -> p a b", b=1)
        nc.vector.tensor_tensor(
            out=cmaxv, in0=T6v[:, :, 0:1], in1=T6v[:, :, 1:2], op=Alu.max
        )

    # ---- parallel bisection of G(t) = sum relu(cmax - t)^4 = 256 ----
    rmax = small.tile([P, 1], F32, tag="rmax")
    lo = small.tile([P, 1], F32, tag="lo")
    nc.vector.reduce_max(out=rmax, in_=rmaxp, axis=mybir.AxisListType.X)
    nc.vector.tensor_scalar(lo, rmax, 4.0, None, op0=Alu.subtract)

    dm = DM0
    for r in range(B_ROUNDS):
        step = float(np.float32(dm / (B_K + 1)))
        gvs = small.tile([P, 8], F32, tag=f"gvs{r}")
        tms = small.tile([P, B_K], F32, tag=f"tms{r}")
        nc.vector.tensor_scalar(
            tms, sv_sb[:, r * B_K : (r + 1) * B_K], lo, None, op0=Alu.add
        )
        for k in range(1, B_K + 1):
            yg = bis.tile([P, CMAX_W], FP16, tag="yg")
            zg = bis.tile([P, CMAX_W], FP16, tag="zg")
            wg = bis.tile([P, CMAX_W], FP16, tag="wg")
            nc.vector.tensor_scalar(
                yg, cmax, tms[:, k - 1 : k], 0.0, op0=Alu.subtract, op1=Alu.max
            )
            nc.vector.tensor_mul(zg, yg, yg)
            nc.vector.scalar_tensor_tensor(
                out=wg, in0=zg, scalar=1.0, in1=zg, op0=Alu.mult, op1=Alu.mult,
                accum_out=gvs[:, k - 1 : k],
            )
        steps = small.tile([P, B_K], F32, tag=f"steps{r}")
        ssum = small.tile([P, 1], F32, tag=f"ssum{r}")
        nc.vector.tensor_scalar(
            steps, gvs[:, 0:B_K], 256.0, step, op0=Alu.is_ge, op1=Alu.mult
        )
        nc.vector.reduce_sum(out=ssum, in_=steps, axis=mybir.AxisListType.X)
        nc.vector.tensor_add(lo, lo, ssum)
        dm = step

    theta = small.tile([P, 1], F32, tag="theta")
    nc.vector.tensor_scalar(theta, lo, -G_MARGIN, None, op0=Alu.add)

    # ---- moment pass at theta: A2, A3, A4 ----
    a2p = small.tile([P, N_CHUNKS], F32, tag="a2p")
    a3p = small.tile([P, N_CHUNKS], F32, tag="a3p")
    a4p = small.tile([P, N_CHUNKS], F32, tag="a4p")
    us = []
    for c in range(N_CHUNKS):
        u_c = xt.tile([P, CHUNK], FP16, tag="xt", name=f"u{c}")
        z_c = wk.tile([P, CHUNK], BF16, tag="wk", name=f"z{c}")
        w3s = wk.tile([P, CHUNK], BF16, tag="wk", name=f"w3{c}")
        w4s = wk.tile([P, CHUNK], BF16, tag="wk", name=f"w4{c}")
        with tc.high_priority():
            nc.vector.tensor_scalar(
                u_c, xts[c], theta, 0.0, op0=Alu.subtract, op1=Alu.max
            )
        nc.scalar.activation(z_c, u_c, Act.Square, accum_out=a2p[:, c : c + 1])
        nc.vector.scalar_tensor_tensor(
            out=w3s, in0=z_c, scalar=1.0, in1=u_c, op0=Alu.mult, op1=Alu.mult,
            accum_out=a3p[:, c : c + 1],
        )
        if c % 3 == 2:
            nc.vector.scalar_tensor_tensor(
                out=w4s, in0=z_c, scalar=1.0, in1=z_c, op0=Alu.mult,
                op1=Alu.mult, accum_out=a4p[:, c : c + 1],
            )
        else:
            nc.scalar.activation(w4s, z_c, Act.Square, accum_out=a4p[:, c : c + 1])
        us.append(u_c)

    a2 = small.tile([P, 1], F32, tag="a2")
    a3 = small.tile([P, 1], F32, tag="a3")
    a4 = small.tile([P, 1], F32, tag="a4")
    for acc, prt in ((a2, a2p), (a3, a3p), (a4, a4p)):
        nc.vector.reduce_sum(out=acc, in_=prt, axis=mybir.AxisListType.X)

    # ---- Newton on P(d) = a4 - k1p d + k2 d^2 = 256, 2 steps ----
    k1p = small.tile([P, 1], F32, tag="k1p")   # 4*A3 = -P'(0)
    k2 = small.tile([P, 1], F32, tag="k2")     # 6*A2
    q2 = small.tile([P, 1], F32, tag="q2")     # 12*A2
    dlt = small.tile([P, 1], F32, tag="dlt")
    pv = small.tile([P, 1], F32, tag="pv")
    ppv = small.tile([P, 1], F32, tag="ppv")
    stp = small.tile([P, 1], F32, tag="stp")
    a4m = small.tile([P, 1], F32, tag="a4m")
    t1 = small.tile([P, 1], F32, tag="t1")

    nc.vector.tensor_scalar(a4m, a4, -256.0, None, op0=Alu.add)
    nc.vector.tensor_scalar(k1p, a3, 4.0, None, op0=Alu.mult)
    nc.vector.tensor_scalar(k2, a2, 6.0, None, op0=Alu.mult)
    nc.vector.tensor_scalar(q2, a2, 12.0, None, op0=Alu.mult)
    # step 1 (closed form from d=0): d1 = (A4-256) / (4*A3)
    nc.vector.reciprocal(ppv, k1p)
    nc.vector.tensor_mul(dlt, a4m, ppv)
    # step 2: pv = a4m - k1p*d + k2*d^2 ; P'(d) = q2*d - k1p (negative);
    # the P' reciprocal runs in parallel with the P evaluation
    nc.vector.tensor_mul(ppv, q2, dlt)
    nc.vector.tensor_mul(t1, k2, dlt)
    nc.vector.tensor_sub(ppv, ppv, k1p)
    nc.vector.tensor_sub(t1, t1, k1p)
    nc.vector.reciprocal(ppv, ppv)
    nc.vector.tensor_mul(pv, t1, dlt)
    nc.vector.tensor_add(pv, pv, a4m)
    nc.vector.tensor_mul(stp, pv, ppv)
    nc.vector.tensor_sub(dlt, dlt, stp)

    # s2 = P(dlt)^(-1/2) with P = a4 - k1p*d + k2*d^2
    s2 = small.tile([P, 1], F32, tag="s2")
    nc.vector.tensor_mul(t1, k2, dlt)
    nc.vector.tensor_sub(t1, t1, k1p)
    nc.vector.tensor_mul(pv, t1, dlt)
    nc.vector.tensor_add(pv, pv, a4)
    nc.vector.reciprocal(s2, pv)
    nc.scalar.activation(s2, s2, Act.Sqrt)

    # ---- output pass: p = (s2 * relu(u - d)^2)^2 ----
    for c in range(N_CHUNKS):
        v_c = wk.tile([P, CHUNK], FP16, tag="wk", name=f"v{c}")
        q_c = wk.tile([P, CHUNK], FP16, tag="wk", name=f"q{c}")
        f_out = fout.tile([P, CHUNK], FP16, tag="fout", name=f"fout{c}")
        with tc.high_priority():
            nc.vector.tensor_scalar(
                v_c, us[c], dlt, 0.0, op0=Alu.subtract, op1=Alu.max
            )
        if c % 2 == 0:
            nc.gpsimd.tensor_mul(q_c, v_c, v_c)
        else:
            nc.vector.tensor_mul(q_c, v_c, v_c)
        nc.scalar.activation(f_out, q_c, Act.Square, scale=s2)
        nc.scalar.dma_start(
            out=out_dram[row0 : row0 + P, c * CHUNK : (c + 1) * CHUNK], in_=f_out
        )


def build_bass():
    from concourse import bacc

    nc = bacc.Bacc(None, target_bir_lowering=False)
    x_dram = nc.dram_tensor("x", [ROWS_PER_CORE, D], F32, kind="ExternalInput")
    sv_dram = nc.dram_tensor("sv", [P, 2 * B_K], F32, kind="ExternalInput")
    out_dram = nc.dram_tensor("out", [ROWS_PER_CORE, D], FP16, kind="ExternalOutput")
    with TileContext(nc) as tc:
        with (
            tc.tile_pool(name="xt", bufs=16) as xt,
            tc.tile_pool(name="fio", bufs=3) as fio,
            tc.tile_pool(name="fout", bufs=2) as fout,
            tc.tile_pool(name="wk", bufs=5) as wk,
            tc.tile_pool(name="tr1", bufs=1) as tr1,
            tc.tile_pool(name="tr2", bufs=1) as tr2,
            tc.tile_pool(name="tr3", bufs=1) as tr3,
            tc.tile_pool(name="t4", bufs=2) as t4p,
            tc.tile_pool(name="trs", bufs=2) as trs,
            tc.tile_pool(name="bis", bufs=2) as bis,
            tc.tile_pool(name="small", bufs=1) as small,
        ):
            pools = (xt, fio, fout, wk, tr1, tr2, tr3, t4p, trs, bis, small)
            sv_sb = small.tile([P, 2 * B_K], F32, tag="sv")
            nc.sync.dma_start(out=sv_sb, in_=sv_dram[:, :])
            for t in range(N_ROW_TILES):
                _row_tile(tc, pools, x_dram, out_dram, sv_sb, t * P, t)
    nc.compile()
    return nc


_NC_CACHE = None


def kernel(input: np.ndarray) -> np.ndarray:
    global _NC_CACHE
    from concourse.bass_utils import run_bass_kernel_spmd

    x = np.ascontiguousarray(input, dtype=np.float32)
    assert x.shape == (ROWS_PER_CORE * N_CORES, D)

    if _NC_CACHE is None:
        _NC_CACHE = build_bass()
    nc = _NC_CACHE

    steps = []
    dm = DM0
    for _ in range(B_ROUNDS):
        st = float(np.float32(dm / (B_K + 1)))
        steps.extend(st * k for k in range(1, B_K + 1))
        dm = st
    sv = np.tile(np.asarray(steps, dtype=np.float32), (P, 1))
    in_maps = [
        {"x": x[i * ROWS_PER_CORE : (i + 1) * ROWS_PER_CORE], "sv": sv}
        for i in range(N_CORES)
    ]
    res = run_bass_kernel_spmd(nc, in_maps, core_ids=list(range(N_CORES)))
    return np.concatenate(
        [r["out"].astype(np.float32) for r in res.results], axis=0
    )


# revision 20
# speedup vs baseline: 1.0142x; 1.0142x over previous
"""Entmax (alpha=1.25) bisection kernel for Trainium2, 8 NeuronCores.

Solves  sum_j relu(x_j - tau)^4 = 256  per row (tau = 4*tau_ref) and emits
p = relu(x - tau)^4 / P.  Per 128-row tile:
  1. stream the row in as f32; GPSIMD down-converts to fp16 (x~); the
     scheme then solves entmax of x~ exactly (the fp16 rounding of x
     contributes ~3e-3 relative error, well inside the 2e-2 gate),
  2. group maxima cmax[250] (groups of 128) via a pairwise tt-max tree on
     fp16 (2x DVE mode), 4 levels per chunk + 3 tile-wide,
  3. bracket tau with 2 rounds of 7-point parallel bisection on
     G(t) = sum relu(cmax - t)^4: the 7 G evaluations per round are
     independent (short critical path); lo += step * #{G_k >= 256}.
     G <= F pointwise so theta = lo - margin is a guaranteed lower bound
     of tau, within d <~ 0.14,
  4. moment pass at theta: u = relu(x~-theta) (DVE 4x ts), z = u^2 + A2
     (ACT square-accum), A3 = sum z*u (DVE STT-accum), A4 = sum z^2 (ACT
     square-accum),
  5. 2 Newton steps on P(d) = A4 - 4*A3 d + 6*A2 d^2 = 256 give
     tau = theta + d and the normalizer P,
  6. output: v = relu(u - d) (== relu(x~-tau) exactly; frees x~ early so
     the next tile's loads overlap this tile's moment phase), q = v^2
     (GPSIMD mul, 3/10 chunks on DVE), p = (s2*q)^2 on ACT with
     s2 = P^-1/2 (f32 out) -> DMA.

Engine totals per core (us): DVE ~180 (tree, bisect, u, v, A3, 3 q),
ACT ~179 (z, A4, p), Pool ~180 (conv, 7 q), DMA ~182 (65.5 MB at
360 B/ns) -- balanced against the DMA roofline.
"""

import numpy as np

import concourse.bass as bass
import concourse.mybir as mybir
from concourse.tile import TileContext

P = 128                    # partitions
D = 32000                  # row length
ROWS_PER_CORE = 256        # 2048 / 8 cores
N_ROW_TILES = ROWS_PER_CORE // P   # 2
N_CORES = 8

CHUNK = 3200               # column chunk (load, moments, output)
N_CHUNKS = D // CHUNK      # 10
CW = 128                   # elements per chunk-max group
NG = CHUNK // CW           # 25 groups per chunk
CMAX_W = D // CW           # 250 group maxima per row tile
B_ROUNDS = 2               # parallel-bisection rounds
B_K = 7                    # candidates per round (3 bits)
G_MARGIN = 0.002           # fp16 G-evaluation safety margin on theta
NEWTON_ITERS = 2

F32 = mybir.dt.float32
FP16 = mybir.dt.float16
BF16 = mybir.dt.bfloat16
DM0 = float(np.float32(4.0 - 4.0 * (1.0 / D) ** 0.25))  # initial bracket width


def _row_tile(tc, pools, x_dram, out_dram, sv_sb, row0, tidx):
    nc = tc.nc
    xt, fio, fout, wk, tr1, tr2, tr3, t4p, trs, bis, small = pools
    Alu = mybir.AluOpType
    Act = mybir.ActivationFunctionType

    # ---- load, fp16 convert (GPSIMD), per-chunk max-tree levels 1-4 ----
    xts = []
    T4 = t4p.tile([P, N_CHUNKS * NG * 8], FP16, tag="t4")   # [P, 2000]
    rmaxp = small.tile([P, N_CHUNKS], F32, tag="rmaxp")
    for c in range(N_CHUNKS):
        f_in = fio.tile([P, CHUNK], F32, tag="fin", name=f"fin{c}")
        nc.sync.dma_start(
            out=f_in, in_=x_dram[row0 : row0 + P, c * CHUNK : (c + 1) * CHUNK]
        )
        x_c = xt.tile([P, CHUNK], FP16, tag="xt", name=f"xt{c}")
        if tidx == 0 and c % 2 == 1:
            nc.vector.tensor_copy(x_c, f_in)
        else:
            nc.gpsimd.tensor_copy(x_c, f_in)
        xts.append(x_c)

        g0 = x_c.rearrange("p (a b) -> p a b", b=CW)            # [P, 25, 128]
        l1 = tr1.tile([P, NG * 64], FP16, tag="tr1")
        l1v = l1.rearrange("p (a b) -> p a b", b=64)
        nc.vector.tensor_tensor(
            out=l1v, in0=g0[:, :, 0:64], in1=g0[:, :, 64:128], op=Alu.max
        )
        l2 = tr2.tile([P, NG * 32], FP16, tag="tr2")
        l2v = l2.rearrange("p (a b) -> p a b", b=32)
        nc.vector.tensor_tensor(
            out=l2v, in0=l1v[:, :, 0:32], in1=l1v[:, :, 32:64], op=Alu.max
        )
        l3 = tr3.tile([P, NG * 16], FP16, tag="tr3")
        l3v = l3.rearrange("p (a b) -> p a b", b=16)
        nc.vector.tensor_tensor(
            out=l3v, in0=l2v[:, :, 0:16], in1=l2v[:, :, 16:32], op=Alu.max
        )
        t4v = T4[:, c * NG * 8 : (c + 1) * NG * 8].rearrange(
            "p (a b) -> p a b", b=8
        )
        nc.vector.tensor_tensor(
            out=t4v, in0=l3v[:, :, 0:8], in1=l3v[:, :, 8:16], op=Alu.max
        )
        nc.vector.reduce_max(
            out=rmaxp[:, c : c + 1],
            in_=T4[:, c * NG * 8 : (c + 1) * NG * 8],
            axis=mybir.AxisListType.X,
        )

    # ---- tile-wide max-tree levels 5-7 -> cmax [P, 250] ----
    T4v = T4.rearrange("p (a b) -> p a b", b=8)                 # [P, 250, 8]
    T5 = trs.tile([P, CMAX_W * 4], FP16, tag="t5")
    T5v = T5.rearrange("p (a b) -> p a b", b=4)
    nc.vector.tensor_tensor(out=T5v, in0=T4v[:, :, 0:4], in1=T4v[:, :, 4:8], op=Alu.max)
    T6 = trs.tile([P, CMAX_W * 2], FP16, tag="t6")
    T6v = T6.rearrange("p (a b) -> p a b", b=2)
    nc.vector.tensor_tensor(out=T6v, in0=T5v[:, :, 0:2], in1=T5v[:, :, 2:4], op=Alu.max)
    cmax = trs.tile([P, CMAX_W], FP16, tag="cmax")
    cmaxv = cmax.rearrange("p (a b) -> p a b", b=1)
    nc.vector.tensor_tensor(
        out=cmaxv, in0=T6v[:, :, 0:1], in1=T6v[:, :, 1:2], op=Alu.max
    )

    # ---- parallel bisection of G(t) = sum relu(cmax - t)^4 = 256 ----
    rmax = small.tile([P, 1], F32, tag="rmax")
    lo = small.tile([P, 1], F32, tag="lo")
    nc.vector.reduce_max(out=rmax, in_=rmaxp, axis=mybir.AxisListType.X)
    nc.vector.tensor_scalar(lo, rmax, 4.0, None, op0=Alu.subtract)

    dm = DM0
    for r in range(B_ROUNDS):
        step = float(np.float32(dm / (B_K + 1)))
        gvs = small.tile([P, 8], F32, tag=f"gvs{r}")
        tms = small.tile([P, B_K], F32, tag=f"tms{r}")
        nc.vector.tensor_scalar(
            tms, sv_sb[:, r * B_K : (r + 1) * B_K], lo, None, op0=Alu.add
        )
        for k in range(1, B_K + 1):
            yg = bis.tile([P, CMAX_W], FP16, tag="yg")
            zg = bis.tile([P, CMAX_W], FP16, tag="zg")
            wg = bis.tile([P, CMAX_W], FP16, tag="wg")
            nc.vector.tensor_scalar(
                yg, cmax, tms[:, k - 1 : k], 0.0, op0=Alu.subtract, op1=Alu.max
            )
            nc.vector.tensor_mul(zg, yg, yg)
            nc.vector.scalar_tensor_tensor(
                out=wg, in0=zg, scalar=1.0, in1=zg, op0=Alu.mult, op1=Alu.mult,
                accum_out=gvs[:, k - 1 : k],
            )
        steps = small.tile([P, B_K], F32, tag=f"steps{r}")
        ssum = small.tile([P, 1], F32, tag=f"ssum{r}")
        nc.vector.tensor_scalar(
            steps, gvs[:, 0:B_K], 256.0, step, op0=Alu.is_ge, op1=Alu.mult
        )
        nc.vector.reduce_sum(out=ssum, in_=steps, axis=mybir.AxisListType.X)
        nc.vector.tensor_add(lo, lo, ssum)
        dm = step

    theta = small.tile([P, 1], F32, tag="theta")
    nc.vector.tensor_scalar(theta, lo, -G_MARGIN, None, op0=Alu.add)

    # ---- moment pass at theta: A2, A3, A4 ----
    a2p = small.tile([P, N_CHUNKS], F32, tag="a2p")
    a3p = small.tile([P, N_CHUNKS], F32, tag="a3p")
    a4p = small.tile([P, N_CHUNKS], F32, tag="a4p")
    us = []
    for c in range(N_CHUNKS):
        u_c = xt.tile([P, CHUNK], FP16, tag="xt", name=f"u{c}")
        z_c = wk.tile([P, CHUNK], BF16, tag="wk", name=f"z{c}")
        w3s = wk.tile([P, CHUNK], BF16, tag="wk", name=f"w3{c}")
        w4s = wk.tile([P, CHUNK], BF16, tag="wk", name=f"w4{c}")
        nc.vector.tensor_scalar(
            u_c, xts[c], theta, 0.0, op0=Alu.subtract, op1=Alu.max
        )
        nc.scalar.activation(z_c, u_c, Act.Square, accum_out=a2p[:, c : c + 1])
        nc.vector.scalar_tensor_tensor(
            out=w3s, in0=z_c, scalar=1.0, in1=u_c, op0=Alu.mult, op1=Alu.mult,
            accum_out=a3p[:, c : c + 1],
        )
        if c % 3 == 2:
            nc.vector.scalar_tensor_tensor(
                out=w4s, in0=z_c, scalar=1.0, in1=z_c, op0=Alu.mult,
                op1=Alu.mult, accum_out=a4p[:, c : c + 1],
            )
        else:
            nc.scalar.activation(w4s, z_c, Act.Square, accum_out=a4p[:, c : c + 1])
        us.append(u_c)

    a2 = small.tile([P, 1], F32, tag="a2")
    a3 = small.tile([P, 1], F32, tag="a3")
    a4 = small.tile([P, 1], F32, tag="a4")
    for acc, prt in ((a2, a2p), (a3, a3p), (a4, a4p)):
        nc.vector.reduce_sum(out=acc, in_=prt, axis=mybir.AxisListType.X)

    # ---- Newton on P(d) = a4 - k1p d + k2 d^2 = 256, 2 steps ----
    k1p = small.tile([P, 1], F32, tag="k1p")   # 4*A3 = -P'(0)
    k2 = small.tile([P, 1], F32, tag="k2")     # 6*A2
    q2 = small.tile([P, 1], F32, tag="q2")     # 12*A2
    dlt = small.tile([P, 1], F32, tag="dlt")
    pv = small.tile([P, 1], F32, tag="pv")
    ppv = small.tile([P, 1], F32, tag="ppv")
    stp = small.tile([P, 1], F32, tag="stp")
    a4m = small.tile([P, 1], F32, tag="a4m")
    t1 = small.tile([P, 1], F32, tag="t1")

    nc.vector.tensor_scalar(a4m, a4, -256.0, None, op0=Alu.add)
    nc.vector.tensor_scalar(k1p, a3, 4.0, None, op0=Alu.mult)
    nc.vector.tensor_scalar(k2, a2, 6.0, None, op0=Alu.mult)
    nc.vector.tensor_scalar(q2, a2, 12.0, None, op0=Alu.mult)
    # step 1 (closed form from d=0): d1 = (A4-256) / (4*A3)
    nc.vector.reciprocal(ppv, k1p)
    nc.vector.tensor_mul(dlt, a4m, ppv)
    # step 2: pv = a4m - k1p*d + k2*d^2 ; P'(d) = q2*d - k1p (negative);
    # the P' reciprocal runs in parallel with the P evaluation
    nc.vector.tensor_mul(ppv, q2, dlt)
    nc.vector.tensor_mul(t1, k2, dlt)
    nc.vector.tensor_sub(ppv, ppv, k1p)
    nc.vector.tensor_sub(t1, t1, k1p)
    nc.vector.reciprocal(ppv, ppv)
    nc.vector.tensor_mul(pv, t1, dlt)
    nc.vector.tensor_add(pv, pv, a4m)
    nc.vector.tensor_mul(stp, pv, ppv)
    nc.vector.tensor_sub(dlt, dlt, stp)

    # s2 = P(dlt)^(-1/2) with P = a4 - k1p*d + k2*d^2
    s2 = small.tile([P, 1], F32, tag="s2")
    nc.vector.tensor_mul(t1, k2, dlt)
    nc.vector.tensor_sub(t1, t1, k1p)
    nc.vector.tensor_mul(pv, t1, dlt)
    nc.vector.tensor_add(pv, pv, a4)
    nc.vector.reciprocal(s2, pv)
    nc.scalar.activation(s2, s2, Act.Sqrt)

    # ---- output pass: p = (s2 * relu(u - d)^2)^2 ----
    for c in range(N_CHUNKS):
        v_c = wk.tile([P, CHUNK], FP16, tag="wk", name=f"v{c}")
        q_c = wk.tile([P, CHUNK], FP16, tag="wk", name=f"q{c}")
        f_out = fout.tile([P, CHUNK], FP16, tag="fout", name=f"fout{c}")
        nc.vector.tensor_scalar(
            v_c, us[c], dlt, 0.0, op0=Alu.subtract, op1=Alu.max
        )
        if c % 2 == 0:
            nc.gpsimd.tensor_mul(q_c, v_c, v_c)
        else:
            nc.vector.tensor_mul(q_c, v_c, v_c)
        nc.scalar.activation(f_out, q_c, Act.Square, scale=s2)
        nc.scalar.dma_start(
            out=out_dram[row0 : row0 + P, c * CHUNK : (c + 1) * CHUNK], in_=f_out
        )


def build_bass():
    from concourse import bacc

    nc = bacc.Bacc(None, target_bir_lowering=False)
    x_dram = nc.dram_tensor("x", [ROWS_PER_CORE, D], F32, kind="ExternalInput")
    sv_dram = nc.dram_tensor("sv", [P, 2 * B_K], F32, kind="ExternalInput")
    out_dram = nc.dram_tensor("out", [ROWS_PER_CORE, D], FP16, kind="ExternalOutput")
    with TileContext(nc) as tc:
        with (
            tc.tile_pool(name="xt", bufs=16) as xt,
            tc.tile_pool(name="fio", bufs=3) as fio,
            tc.tile_pool(name="fout", bufs=2) as fout,
            tc.tile_pool(name="wk", bufs=5) as wk,
            tc.tile_pool(name="tr1", bufs=1) as tr1,
            tc.tile_pool(name="tr2", bufs=1) as tr2,
            tc.tile_pool(name="tr3", bufs=1) as tr3,
            tc.tile_pool(name="t4", bufs=2) as t4p,
            tc.tile_pool(name="trs", bufs=2) as trs,
            tc.tile_pool(name="bis", bufs=2) as bis,
            tc.tile_pool(name="small", bufs=1) as small,
        ):
            pools = (xt, fio, fout, wk, tr1, tr2, tr3, t4p, trs, bis, small)
            sv_sb = small.tile([P, 2 * B_K], F32, tag="sv")
            nc.sync.dma_start(out=sv_sb, in_=sv_dram[:, :])
            for t in range(N_ROW_TILES):
                _row_tile(tc, pools, x_dram, out_dram, sv_sb, t * P, t)
    nc.compile()
    return nc


_NC_CACHE = None


def kernel(input: np.ndarray) -> np.ndarray:
    global _NC_CACHE
    from concourse.bass_utils import run_bass_kernel_spmd

    x = np.ascontiguousarray(input, dtype=np.float32)
    assert x.shape == (ROWS_PER_CORE * N_CORES, D)

    if _NC_CACHE is None:
        _NC_CACHE = build_bass()
    nc = _NC_CACHE

    steps = []
    dm = DM0
    for _ in range(B_ROUNDS):
        st = float(np.float32(dm / (B_K + 1)))
        steps.extend(st * k for k in range(1, B_K + 1))
        dm = st
    sv = np.tile(np.asarray(steps, dtype=np.float32), (P, 1))
    in_maps = [
        {"x": x[i * ROWS_PER_CORE : (i + 1) * ROWS_PER_CORE], "sv": sv}
        for i in range(N_CORES)
    ]
    res = run_bass_kernel_spmd(nc, in_maps, core_ids=list(range(N_CORES)))
    return np.concatenate(
        [r["out"].astype(np.float32) for r in res.results], axis=0
    )
